# revision 5
# baseline (speedup 1.0000x reference)
"""MoE gate kernel for Trainium2 (8 NeuronCores, SPMD token-parallel).

Computes, for hidden_states [4, 8192, 2048] and gate weight [64, 2048]:
  logits = x @ W.T        (f32)
  scores = softmax(logits)
  topk_weight, topk_idx = top_k(scores, 8)   (descending)
  aux_loss = sum(Pi * ce * 64) * 0.01
Sharding: token dim split across 8 cores (4096 tokens each); the tiny gate
weight is replicated (passed pre-transposed [2048, 64]). Per-core partial
sums of scores (Pi) come back with the outputs; counts/aux are reduced on
host from the gathered shards.

Per-core pipeline (Tile framework):
  DMA x tile [128 tok, 2048] (natural layout, full-bandwidth)
  -> PE transpose 16x [128,128] chunks (fp32 via identity)
  -> evacuate PSUM->SBUF (alternating Scalar/Vector engines)
  -> 16 accumulating fp32 matmuls against W.T chunks -> logits [128 tok, 64]
  -> ScalarE Exp with accumulated row sums -> VectorE reciprocal
  -> VectorE max8/max_index top-8, scale by 1/sum
  -> PE [1,64] matmul accumulates Pi partial (recip^T @ exp)
"""

import numpy as np

import concourse.bass as bass
import concourse.mybir as mybir
import concourse.tile as tile
from concourse.bass_utils import run_bass_kernel_spmd
from concourse.masks import make_identity
from concourse.vector_clock import ScopedClock

N_CORES = 8
B, S, D = 4, 8192, 2048
N = B * S            # 32768 tokens
TOK = N // N_CORES   # 4096 tokens per core
E = 64               # experts
K = 8                # top-k
P = 128              # partitions
C = D // P           # 16 contraction chunks
NT = TOK // P        # 32 token tiles per core
ALPHA = 0.01

F32 = mybir.dt.float32
U32 = mybir.dt.uint32


import bass_rust


def _split_multi_waits(nc):
    """walrus in this container lowers at most one sync wait per instruction;
    hoist extra waits onto injected same-engine NoOps placed just before."""
    n = 0
    for fn in nc.m.functions:
        for bb in fn.blocks:
            insts = list(bb.instructions)
            out = []
            for inst in insts:
                si = inst.sync_info
                if si is not None and len(si.on_wait) > 1:
                    waits = list(si.on_wait)
                    for w in waits[:-1]:
                        nop = bass_rust.InstNoOp(name=f"waitfix-{n}")
                        n += 1
                        nop.engine = inst.engine
                        nop.sync_info = mybir.SyncInfo(on_wait=[w], on_update=[])
                        out.append(nop)
                    inst.sync_info = mybir.SyncInfo(
                        on_wait=[waits[-1]], on_update=list(si.on_update)
                    )
                out.append(inst)
            if n:
                bb.instructions = out


def _build_program():
    nc = bass.Bass()
    x = nc.dram_tensor("x", [TOK, D], F32, kind="ExternalInput")
    wt = nc.dram_tensor("wt", [D, E], F32, kind="ExternalInput")
    out_w = nc.dram_tensor("out_w", [TOK, K], F32, kind="ExternalOutput")
    out_idx = nc.dram_tensor("out_idx", [TOK, K], U32, kind="ExternalOutput")
    out_pi = nc.dram_tensor("out_pi", [1, E], F32, kind="ExternalOutput")

    with tile.TileContext(nc) as tc:
        with (
            tc.tile_pool(name="const", bufs=1) as const_pool,
            tc.tile_pool(name="xin", bufs=3) as xin_pool,
            tc.tile_pool(name="xt", bufs=2) as xt_pool,
            tc.tile_pool(name="sm", bufs=4) as sm_pool,
            tc.tile_pool(name="tp_ps", bufs=4, space="PSUM") as tp_psum,
            tc.tile_pool(name="lg_ps", bufs=2, space="PSUM") as lg_psum,
            tc.tile_pool(name="pi_ps", bufs=1, space="PSUM") as pi_psum,
        ):
            wt_sb = const_pool.tile([P, C, E], F32)
            nc.sync.dma_start(wt_sb[:], wt.rearrange("(c p) e -> p c e", p=P))
            ident = const_pool.tile([P, P], F32)
            make_identity(nc, ident)
            pi_acc = const_pool.tile([1, E], F32)
            nc.vector.memset(pi_acc[:], 0.0)

            for i in range(NT):
                xin = xin_pool.tile([P, D], F32)
                nc.sync.dma_start(xin[:], x[i * P : (i + 1) * P, :])

                xt = xt_pool.tile([P, C, P], F32)
                for c in range(C):
                    tp = tp_psum.tile([P, P], F32)
                    nc.tensor.transpose(tp[:], xin[:, c * P : (c + 1) * P], ident[:])
                    if c % 2 == 0:
                        nc.scalar.copy(xt[:, c, :], tp[:])
                    else:
                        nc.vector.tensor_copy(xt[:, c, :], tp[:])

                lg = lg_psum.tile([P, E], F32)
                for c in range(C):
                    nc.tensor.matmul(
                        lg[:],
                        xt[:, c, :],
                        wt_sb[:, c, :],
                        start=(c == 0),
                        stop=(c == C - 1),
                    )

                expt = sm_pool.tile([P, E], F32, tag="expt")
                sums = sm_pool.tile([P, 1], F32, tag="sums")
                nc.scalar.activation(
                    expt[:], lg[:], mybir.ActivationFunctionType.Exp,
                    accum_out=sums[:],
                )
                recip = sm_pool.tile([P, 1], F32, tag="recip")
                nc.vector.reciprocal(recip[:], sums[:])

                vals = sm_pool.tile([P, K], F32, tag="vals")
                idxs = sm_pool.tile([P, K], U32, tag="idxs")
                nc.vector.max(vals[:], expt[:])
                nc.vector.max_index(idxs[:], vals[:], expt[:])
                wout = sm_pool.tile([P, K], F32, tag="wout")
                nc.vector.tensor_scalar_mul(wout[:], vals[:], recip[:])

                pi_ps = pi_psum.tile([1, E], F32)
                nc.tensor.matmul(pi_ps[:], recip[:], expt[:])
                nc.vector.tensor_add(pi_acc[:], pi_acc[:], pi_ps[:])

                nc.sync.dma_start(out_w[i * P : (i + 1) * P, :], wout[:])
                nc.sync.dma_start(out_idx[i * P : (i + 1) * P, :], idxs[:])

            nc.sync.dma_start(out_pi[:], pi_acc[:])
    _split_multi_waits(nc)
    return nc


_PROG = None


LAST_RESULT = None


def _run_spmd(x_full, wt_full, **spmd_kwargs):
    global _PROG, LAST_RESULT
    if _PROG is None:
        _PROG = _build_program()
    in_maps = [
        {"x": x_full[c * TOK : (c + 1) * TOK], "wt": wt_full}
        for c in range(N_CORES)
    ]
    res = run_bass_kernel_spmd(_PROG, in_maps, list(range(N_CORES)), **spmd_kwargs)
    LAST_RESULT = res
    return res


def kernel(hidden_states, weight, _spmd_kwargs=None):
    x = np.ascontiguousarray(
        np.asarray(hidden_states, dtype=np.float32).reshape(N, D)
    )
    w = np.asarray(weight, dtype=np.float32)
    wt_full = np.ascontiguousarray(w.T)  # [D, E]

    res = _run_spmd(x, wt_full, **(_spmd_kwargs or {}))

    idx = np.concatenate(
        [res.results[c]["out_idx"].view(np.int32) for c in range(N_CORES)], axis=0
    )
    tw = np.concatenate(
        [res.results[c]["out_w"] for c in range(N_CORES)], axis=0
    )
    pi_sum = np.zeros(E, dtype=np.float32)
    for c in range(N_CORES):
        pi_sum += res.results[c]["out_pi"][0]

    counts = np.bincount(idx.reshape(-1), minlength=E).astype(np.float32)
    ce = counts / np.float32(idx.size)
    Pi = (pi_sum / np.float32(N)).astype(np.float32)
    aux = np.array(np.sum(Pi * ce) * E * ALPHA, dtype=np.float32)
    return idx, tw, aux


# revision 6
# speedup vs baseline: 1.0024x; 1.0024x over previous
"""MoE gate kernel for Trainium2 (8 NeuronCores, SPMD token-parallel).

For hidden_states [4, 8192, 2048] f32 and gate weight [64, 2048] f32:
  logits = x @ W.T ; scores = softmax(logits)
  topk_weight, topk_idx = top_k(scores, 8)   (descending)
  aux_loss = sum(Pi * ce * 64) * 0.01
Returns (topk_idx int32 [32768,8], topk_weight f32 [32768,8], aux_loss f32).

Sharding: token dim split across 8 cores (4096 tokens each); the tiny gate
weight is replicated (host pre-splits it into fp16 hi/lo pairs). Per-core
score sums (Pi partials) return with the outputs; counts and the aux-loss
scalar are reduced on host from the gathered shards.

Per-core pipeline (Tile framework, all engines overlapped):
  DMA x in 4-tile blocks [128 tok, 4, 2048] (natural layout, full bandwidth)
  -> PE transposes x in [128,128] chunks (fp32, via identity) into PSUM
  -> split-precision evacuation: ScalarE writes hi = fp16(2048*x^T),
     VectorE writes lo = fp16(2048*x^T - hi)   (exact fp16 pair, scaled
     by 2^11 so both parts stay in fp16 normal range)
  -> per 128-chunk: two fp16 matmuls accumulate
        lg[:, :64] += hi @ whi ; lg[:, 64:] += hi @ wlo_s ; lg[:, :64] += lo @ whi
     where whi = fp16(W.T), wlo_s = fp16((W.T - whi) * 2048). All products
     are exact (11-bit mantissas); logits come out at f32 accumulation
     accuracy, ~6x faster than the fp32 LOW/HIGH matmul path.
  -> logits*2048 = lg[:,:64] + lg[:,64:]/2048 (DVE), exp with scale 1/2048
     and accumulated row sums (ScalarE), reciprocal, max8/max_index top-8
  -> merged [weights | idx] tile per 4 tiles, single DMA out
"""

import numpy as np

import bass_rust
import concourse.bass as bass
import concourse.mybir as mybir
import concourse.tile as tile
from concourse.bass_utils import run_bass_kernel_spmd

N_CORES = 8
B, S, D = 4, 8192, 2048
N = B * S            # 32768 tokens
TOK = N // N_CORES   # 4096 tokens per core
E = 64               # experts
K = 8                # top-k
P = 128              # partitions
C = D // P           # 16 contraction chunks
NT = TOK // P        # 32 token tiles per core
ALPHA = 0.01

F32 = mybir.dt.float32
F16 = mybir.dt.float16
U32 = mybir.dt.uint32

TB = 4               # token tiles per DMA block
NB = NT // TB
G = 4                # transposes per PSUM bank group
SC = 2048.0          # 2^11 scale keeping fp16 split parts normal
FP16_MIN_NORMAL = 6.104e-05


def _split_multi_waits(nc):
    """walrus in this container lowers at most one sync wait per instruction;
    hoist extra waits onto injected same-engine NoOps placed just before."""
    n = 0
    for fn in nc.m.functions:
        for bb in fn.blocks:
            insts = list(bb.instructions)
            out = []
            for inst in insts:
                si = inst.sync_info
                if si is not None and len(si.on_wait) > 1:
                    waits = list(si.on_wait)
                    for w in waits[:-1]:
                        nop = bass_rust.InstNoOp(name=f"waitfix-{n}")
                        n += 1
                        nop.engine = inst.engine
                        nop.sync_info = mybir.SyncInfo(on_wait=[w], on_update=[])
                        out.append(nop)
                    inst.sync_info = mybir.SyncInfo(
                        on_wait=[waits[-1]], on_update=list(si.on_update)
                    )
                out.append(inst)
            if n:
                bb.instructions = out


def build_program():
    nc = bass.Bass()
    x = nc.dram_tensor("x", [TOK, D], F32, kind="ExternalInput")
    ident_in = nc.dram_tensor("ident", [P, P], F32, kind="ExternalInput")
    # packed fp16 weights [P, C, 2, E]: whi at [:,:,0,:], wlo_s at [:,:,1,:]
    wt = nc.dram_tensor("wt", [P, C * 2 * E], F16, kind="ExternalInput")
    # merged output per token: [:, 0:8] weights f32, [:, 8:16] idx u32 bits
    out_m = nc.dram_tensor("out_m", [TOK, 2 * K], F32, kind="ExternalOutput")
    out_pi = nc.dram_tensor("out_pi", [P, E], F32, kind="ExternalOutput")

    with tile.TileContext(nc) as tc:
        with (
            tc.tile_pool(name="const", bufs=1) as const_pool,
            tc.tile_pool(name="xin", bufs=3) as xin_pool,
            tc.tile_pool(name="xh", bufs=2) as xh_pool,
            tc.tile_pool(name="sm", bufs=6) as sm_pool,
            tc.tile_pool(name="tp_ps", bufs=6, space="PSUM") as tp_psum,
            tc.tile_pool(name="lg_ps", bufs=2, space="PSUM") as lg_psum,
        ):
            ident = const_pool.tile([P, P], F32)
            nc.sync.dma_start(ident[:], ident_in[:])
            wt_sb = const_pool.tile([P, C, 2, E], F16)
            nc.sync.dma_start(wt_sb[:].rearrange("p c h e -> p (c h e)"), wt[:])
            sexp = const_pool.tile([P, E], F32)
            nc.vector.memset(sexp[:], 0.0)
            warm = const_pool.tile([1, 1], F32)
            nc.vector.memset(warm[:], 0.0)
            nc.scalar.activation(warm[:], warm[:], mybir.ActivationFunctionType.Exp)

            for ib in range(NB):
                xin = xin_pool.tile([P, TB, D], F32)
                if ib == 0:
                    # smaller first transfers so the PE starts sooner
                    nc.sync.dma_start(xin[:, 0, : D // 2], x[:P, : D // 2])
                    nc.sync.dma_start(xin[:, 0, D // 2 :], x[:P, D // 2 :])
                    for b0 in range(1, TB):
                        nc.sync.dma_start(xin[:, b0, :], x[b0 * P : (b0 + 1) * P, :])
                else:
                    nc.sync.dma_start(
                        xin[:], x[ib * TB * P : (ib + 1) * TB * P, :].rearrange(
                            "(b p) d -> p b d", p=P
                        )
                    )
                for b in range(TB):
                    i = ib * TB + b
                    xh = xh_pool.tile([P, C, P], F16, tag="xh")
                    xl = xh_pool.tile([P, C, P], F16, tag="xl")
                    for g in range(C // G):
                        tp = tp_psum.tile([P, G * P], F32)
                        for k in range(G):
                            c = g * G + k
                            nc.tensor.transpose(
                                tp[:, k * P : (k + 1) * P],
                                xin[:, b, c * P : (c + 1) * P],
                                ident[:],
                            )
                        hi = xh[:, g * G : (g + 1) * G, :].rearrange("p c t -> p (c t)")
                        lo = xl[:, g * G : (g + 1) * G, :].rearrange("p c t -> p (c t)")
                        nc.scalar.activation(
                            hi, tp[:], mybir.ActivationFunctionType.Copy, scale=SC
                        )
                        nc.vector.scalar_tensor_tensor(
                            lo, tp[:], SC, hi,
                            op0=mybir.AluOpType.mult,
                            op1=mybir.AluOpType.subtract,
                        )

                    lg = lg_psum.tile([P, 2 * E], F32)
                    for c in range(C):
                        nc.tensor.matmul(
                            lg[:],
                            xh[:, c, :],
                            wt_sb[:, c, :, :].rearrange("p h e -> p (h e)"),
                            start=(c == 0),
                            stop=False,
                        )
                        nc.tensor.matmul(
                            lg[:, :E],
                            xl[:, c, :],
                            wt_sb[:, c, 0, :],
                            start=False,
                            stop=(c == C - 1),
                        )

                    # logits*2048 = lg[:, :64] + lg[:, 64:]/2048 (one PSUM
                    # operand per DVE op: bounce lgB through SBUF on ACT)
                    lgb = sm_pool.tile([P, E], F32, tag="lgb")
                    nc.scalar.copy(lgb[:], lg[:, E:])
                    lt = sm_pool.tile([P, E], F32, tag="lt")
                    nc.vector.scalar_tensor_tensor(
                        lt[:], lgb[:], 1.0 / SC, lg[:, :E],
                        op0=mybir.AluOpType.mult,
                        op1=mybir.AluOpType.add,
                    )
                    expt = sm_pool.tile([P, E], F32, tag="expt")
                    sums = sm_pool.tile([P, 1], F32, tag="sums")
                    nc.scalar.activation(
                        expt[:], lt[:], mybir.ActivationFunctionType.Exp,
                        scale=float(1.0 / SC), accum_out=sums[:],
                    )
                    recip = sm_pool.tile([P, 1], F32, tag="recip")
                    nc.vector.reciprocal(recip[:], sums[:])

                    if b == 0:
                        merged = sm_pool.tile([P, TB, 2 * K], F32, tag="merged")
                    vals = sm_pool.tile([P, K], F32, tag="vals")
                    nc.vector.max(vals[:], expt[:])
                    nc.vector.max_index(
                        merged[:, b, K:].bitcast(U32), vals[:], expt[:]
                    )
                    nc.scalar.activation(
                        merged[:, b, :K], vals[:], mybir.ActivationFunctionType.Copy,
                        scale=recip[:],
                    )

                    scl = sm_pool.tile([P, E], F32, tag="scl")
                    nc.vector.tensor_scalar_mul(scl[:], expt[:], recip[:])
                    nc.vector.tensor_add(sexp[:], sexp[:], scl[:])

                    if b == TB - 1:
                        nc.sync.dma_start(
                            out_m[ib * TB * P : (ib + 1) * TB * P, :].rearrange(
                                "(b p) k -> p b k", p=P
                            ),
                            merged[:],
                        )

            nc.sync.dma_start(out_pi[:], sexp[:])
    _split_multi_waits(nc)
    return nc


_PROG = None
LAST_RESULT = None


def _pack_weights(w):
    """w [E, D] f32 -> packed fp16 [P, C*2*E] (whi | wlo_s per chunk),
    subnormals flushed host-side so the residual folds into wlo_s."""
    wt = np.ascontiguousarray(w.T.astype(np.float32))          # [D, E]
    whi = wt.astype(np.float16)
    whi[np.abs(whi) < FP16_MIN_NORMAL] = 0
    wlo_s = ((wt - whi.astype(np.float32)) * SC).astype(np.float16)
    wlo_s[np.abs(wlo_s) < FP16_MIN_NORMAL] = 0
    packed = np.stack(
        [whi.reshape(C, P, E), wlo_s.reshape(C, P, E)], axis=2
    )  # [C, P, 2, E]
    return np.ascontiguousarray(
        packed.transpose(1, 0, 2, 3).reshape(P, C * 2 * E)
    )


def _run_spmd(x_full, wt_full, **spmd_kwargs):
    global _PROG, LAST_RESULT
    if _PROG is None:
        _PROG = build_program()
    ident = np.eye(P, dtype=np.float32)
    in_maps = [
        {"x": x_full[c * TOK : (c + 1) * TOK], "wt": wt_full, "ident": ident}
        for c in range(N_CORES)
    ]
    res = run_bass_kernel_spmd(_PROG, in_maps, list(range(N_CORES)), **spmd_kwargs)
    LAST_RESULT = res
    return res


def kernel(hidden_states, weight, _spmd_kwargs=None):
    x = np.ascontiguousarray(
        np.asarray(hidden_states, dtype=np.float32).reshape(N, D)
    )
    wt_full = _pack_weights(np.asarray(weight, dtype=np.float32))

    res = _run_spmd(x, wt_full, **(_spmd_kwargs or {}))

    merged = np.concatenate(
        [res.results[c]["out_m"] for c in range(N_CORES)], axis=0
    )
    tw = np.ascontiguousarray(merged[:, :K])
    idx = merged[:, K:].view(np.uint32).astype(np.int32)
    pi_sum = np.zeros(E, dtype=np.float32)
    for c in range(N_CORES):
        pi_sum += res.results[c]["out_pi"].sum(axis=0, dtype=np.float32)

    counts = np.bincount(idx.reshape(-1), minlength=E).astype(np.float32)
    ce = counts / np.float32(idx.size)
    Pi = (pi_sum / np.float32(N)).astype(np.float32)
    aux = np.array(np.sum(Pi * ce) * E * ALPHA, dtype=np.float32)
    return idx, tw, aux


# revision 7
# speedup vs baseline: 1.0070x; 1.0046x over previous
"""MoE gate kernel for Trainium2 (8 NeuronCores, SPMD token-parallel).

For hidden_states [4, 8192, 2048] f32 and gate weight [64, 2048] f32:
  logits = x @ W.T ; scores = softmax(logits)
  topk_weight, topk_idx = top_k(scores, 8)   (descending)
  aux_loss = sum(Pi * ce * 64) * 0.01
Returns (topk_idx int32 [32768,8], topk_weight f32 [32768,8], aux_loss f32).

Sharding: token dim split across 8 cores (4096 tokens each); the tiny gate
weight is replicated (host pre-splits it into fp16 hi/lo pairs). Per-core
score sums (Pi partials) return with the outputs; counts and the aux-loss
scalar are reduced on host from the gathered shards.

Per-core pipeline (Tile framework, all engines overlapped):
  DMA x in 4-tile blocks [128 tok, 4, 2048] (natural layout, full bandwidth)
  -> PE transposes x in [128,128] chunks (fp32, via identity) into PSUM
  -> split-precision evacuation: ScalarE writes hi = fp16(2048*x^T),
     VectorE writes lo = fp16(2048*x^T - hi)   (exact fp16 pair, scaled
     by 2^11 so both parts stay in fp16 normal range)
  -> per 128-chunk: two fp16 matmuls accumulate
        lg[:, :64] += hi @ whi ; lg[:, 64:] += hi @ wlo_s ; lg[:, :64] += lo @ whi
     where whi = fp16(W.T), wlo_s = fp16((W.T - whi) * 2048). All products
     are exact (11-bit mantissas); logits come out at f32 accumulation
     accuracy, ~6x faster than the fp32 LOW/HIGH matmul path.
  -> logits*2048 = lg[:,:64] + lg[:,64:]/2048 (DVE), exp with scale 1/2048
     and accumulated row sums (ScalarE), reciprocal, max8/max_index top-8
  -> merged [weights | idx] tile per 4 tiles, single DMA out
"""

import numpy as np

import bass_rust
import concourse.bass as bass
import concourse.mybir as mybir
import concourse.tile as tile
from concourse.bass_utils import run_bass_kernel_spmd

N_CORES = 8
B, S, D = 4, 8192, 2048
N = B * S            # 32768 tokens
TOK = N // N_CORES   # 4096 tokens per core
E = 64               # experts
K = 8                # top-k
P = 128              # partitions
C = D // P           # 16 contraction chunks
NT = TOK // P        # 32 token tiles per core
ALPHA = 0.01

F32 = mybir.dt.float32
F16 = mybir.dt.float16
U32 = mybir.dt.uint32

TB = 4               # token tiles per DMA block
NB = NT // TB
G = 4                # transposes per PSUM bank group
SC = 2048.0          # 2^11 scale keeping fp16 split parts normal
FP16_MIN_NORMAL = 6.104e-05


def _split_multi_waits(nc):
    """walrus in this container lowers at most one sync wait per instruction;
    hoist extra waits onto injected same-engine NoOps placed just before."""
    n = 0
    for fn in nc.m.functions:
        for bb in fn.blocks:
            insts = list(bb.instructions)
            out = []
            for inst in insts:
                si = inst.sync_info
                if si is not None and len(si.on_wait) > 1:
                    waits = list(si.on_wait)
                    for w in waits[:-1]:
                        nop = bass_rust.InstNoOp(name=f"waitfix-{n}")
                        n += 1
                        nop.engine = inst.engine
                        nop.sync_info = mybir.SyncInfo(on_wait=[w], on_update=[])
                        out.append(nop)
                    inst.sync_info = mybir.SyncInfo(
                        on_wait=[waits[-1]], on_update=list(si.on_update)
                    )
                out.append(inst)
            if n:
                bb.instructions = out


def build_program():
    nc = bass.Bass()
    x = nc.dram_tensor("x", [TOK, D], F32, kind="ExternalInput")
    ident_in = nc.dram_tensor("ident", [P, P], F32, kind="ExternalInput")
    # packed fp16 weights [P, C, 2, E]: whi at [:,:,0,:], wlo_s at [:,:,1,:]
    wt = nc.dram_tensor("wt", [P, C * 2 * E], F16, kind="ExternalInput")
    # merged output per token: [:, 0:8] weights f32, [:, 8:16] idx u32 bits
    out_m = nc.dram_tensor("out_m", [TOK, 2 * K], F32, kind="ExternalOutput")
    out_pi = nc.dram_tensor("out_pi", [P, E], F32, kind="ExternalOutput")

    with tile.TileContext(nc) as tc:
        with (
            tc.tile_pool(name="const", bufs=1) as const_pool,
            tc.tile_pool(name="xin", bufs=3) as xin_pool,
            tc.tile_pool(name="xh", bufs=3) as xh_pool,
            tc.tile_pool(name="sm", bufs=6) as sm_pool,
            tc.tile_pool(name="tp_ps", bufs=6, space="PSUM") as tp_psum,
            tc.tile_pool(name="lg_ps", bufs=2, space="PSUM") as lg_psum,
        ):
            ident = const_pool.tile([P, P], F32)
            nc.sync.dma_start(ident[:], ident_in[:])
            wt_sb = const_pool.tile([P, C, 2, E], F16)
            nc.sync.dma_start(wt_sb[:].rearrange("p c h e -> p (c h e)"), wt[:])
            sexp = const_pool.tile([P, E], F32)
            nc.vector.memset(sexp[:], 0.0)
            warm = const_pool.tile([1, 1], F32)
            nc.vector.memset(warm[:], 0.0)
            nc.scalar.activation(warm[:], warm[:], mybir.ActivationFunctionType.Exp)

            for ib in range(NB):
                xin = xin_pool.tile([P, TB, D], F32)
                if ib == 0:
                    # smaller first transfers so the PE starts sooner
                    nc.sync.dma_start(xin[:, 0, : D // 2], x[:P, : D // 2])
                    nc.sync.dma_start(xin[:, 0, D // 2 :], x[:P, D // 2 :])
                    for b0 in range(1, TB):
                        nc.sync.dma_start(xin[:, b0, :], x[b0 * P : (b0 + 1) * P, :])
                else:
                    nc.sync.dma_start(
                        xin[:], x[ib * TB * P : (ib + 1) * TB * P, :].rearrange(
                            "(b p) d -> p b d", p=P
                        )
                    )
                for b in range(TB):
                    i = ib * TB + b
                    xh = xh_pool.tile([P, C, P], F16, tag="xh")
                    xl = xh_pool.tile([P, C, P], F16, tag="xl")
                    for g in range(C // G):
                        tp = tp_psum.tile([P, G * P], F32)
                        for k in range(G):
                            c = g * G + k
                            nc.tensor.transpose(
                                tp[:, k * P : (k + 1) * P],
                                xin[:, b, c * P : (c + 1) * P],
                                ident[:],
                            )
                        hi = xh[:, g * G : (g + 1) * G, :].rearrange("p c t -> p (c t)")
                        lo = xl[:, g * G : (g + 1) * G, :].rearrange("p c t -> p (c t)")
                        nc.scalar.activation(
                            hi, tp[:], mybir.ActivationFunctionType.Copy, scale=SC
                        )
                        nc.vector.scalar_tensor_tensor(
                            lo, tp[:], SC, hi,
                            op0=mybir.AluOpType.mult,
                            op1=mybir.AluOpType.subtract,
                        )

                    lg = lg_psum.tile([P, 2 * E], F32)
                    for c in range(C):
                        nc.tensor.matmul(
                            lg[:],
                            xh[:, c, :],
                            wt_sb[:, c, :, :].rearrange("p h e -> p (h e)"),
                            start=(c == 0),
                            stop=False,
                        )
                        nc.tensor.matmul(
                            lg[:, :E],
                            xl[:, c, :],
                            wt_sb[:, c, 0, :],
                            start=False,
                            stop=(c == C - 1),
                        )

                    # logits*2048 = lg[:, :64] + lg[:, 64:]/2048 (one PSUM
                    # operand per DVE op: bounce lgB through SBUF on ACT)
                    lgb = sm_pool.tile([P, E], F32, tag="lgb")
                    nc.scalar.copy(lgb[:], lg[:, E:])
                    lt = sm_pool.tile([P, E], F32, tag="lt")
                    nc.vector.scalar_tensor_tensor(
                        lt[:], lgb[:], 1.0 / SC, lg[:, :E],
                        op0=mybir.AluOpType.mult,
                        op1=mybir.AluOpType.add,
                    )
                    expt = sm_pool.tile([P, E], F32, tag="expt")
                    sums = sm_pool.tile([P, 1], F32, tag="sums")
                    nc.scalar.activation(
                        expt[:], lt[:], mybir.ActivationFunctionType.Exp,
                        scale=float(1.0 / SC), accum_out=sums[:],
                    )
                    recip = sm_pool.tile([P, 1], F32, tag="recip")
                    nc.vector.reciprocal(recip[:], sums[:])

                    if b == 0:
                        merged = sm_pool.tile([P, TB, 2 * K], F32, tag="merged")
                    vals = sm_pool.tile([P, K], F32, tag="vals")
                    nc.vector.max(vals[:], expt[:])
                    nc.vector.max_index(
                        merged[:, b, K:].bitcast(U32), vals[:], expt[:]
                    )
                    nc.scalar.activation(
                        merged[:, b, :K], vals[:], mybir.ActivationFunctionType.Copy,
                        scale=recip[:],
                    )

                    scl = sm_pool.tile([P, E], F32, tag="scl")
                    nc.vector.tensor_scalar_mul(scl[:], expt[:], recip[:])
                    nc.vector.tensor_add(sexp[:], sexp[:], scl[:])

                    if b == TB - 1:
                        nc.sync.dma_start(
                            out_m[ib * TB * P : (ib + 1) * TB * P, :].rearrange(
                                "(b p) k -> p b k", p=P
                            ),
                            merged[:],
                        )

            nc.sync.dma_start(out_pi[:], sexp[:])
    _split_multi_waits(nc)
    return nc


_PROG = None
LAST_RESULT = None


def _pack_weights(w):
    """w [E, D] f32 -> packed fp16 [P, C*2*E] (whi | wlo_s per chunk),
    subnormals flushed host-side so the residual folds into wlo_s."""
    wt = np.ascontiguousarray(w.T.astype(np.float32))          # [D, E]
    whi = wt.astype(np.float16)
    whi[np.abs(whi) < FP16_MIN_NORMAL] = 0
    wlo_s = ((wt - whi.astype(np.float32)) * SC).astype(np.float16)
    wlo_s[np.abs(wlo_s) < FP16_MIN_NORMAL] = 0
    packed = np.stack(
        [whi.reshape(C, P, E), wlo_s.reshape(C, P, E)], axis=2
    )  # [C, P, 2, E]
    return np.ascontiguousarray(
        packed.transpose(1, 0, 2, 3).reshape(P, C * 2 * E)
    )


def _run_spmd(x_full, wt_full, **spmd_kwargs):
    global _PROG, LAST_RESULT
    if _PROG is None:
        _PROG = build_program()
    ident = np.eye(P, dtype=np.float32)
    in_maps = [
        {"x": x_full[c * TOK : (c + 1) * TOK], "wt": wt_full, "ident": ident}
        for c in range(N_CORES)
    ]
    res = run_bass_kernel_spmd(_PROG, in_maps, list(range(N_CORES)), **spmd_kwargs)
    LAST_RESULT = res
    return res


def kernel(hidden_states, weight, _spmd_kwargs=None):
    x = np.ascontiguousarray(
        np.asarray(hidden_states, dtype=np.float32).reshape(N, D)
    )
    wt_full = _pack_weights(np.asarray(weight, dtype=np.float32))

    res = _run_spmd(x, wt_full, **(_spmd_kwargs or {}))

    merged = np.concatenate(
        [res.results[c]["out_m"] for c in range(N_CORES)], axis=0
    )
    tw = np.ascontiguousarray(merged[:, :K])
    idx = merged[:, K:].view(np.uint32).astype(np.int32)
    pi_sum = np.zeros(E, dtype=np.float32)
    for c in range(N_CORES):
        pi_sum += res.results[c]["out_pi"].sum(axis=0, dtype=np.float32)

    counts = np.bincount(idx.reshape(-1), minlength=E).astype(np.float32)
    ce = counts / np.float32(idx.size)
    Pi = (pi_sum / np.float32(N)).astype(np.float32)
    aux = np.array(np.sum(Pi * ce) * E * ALPHA, dtype=np.float32)
    return idx, tw, aux
